# revision 13
# baseline (speedup 1.0000x reference)
"""Trainium2 Bass kernel for nn_LoRAExpert (moe_routing).

Per token t (expert e_t from contiguous group_sizes, adapter a_t):

    out[t] = x[t] @ W[e_t] + s_{a_t} * (x[t] @ A[a_t, e_t]) @ B[a_t, e_t]

Strategy (expert-parallel over 8 NeuronCores):
  - Host routes tokens: x is already expert-sorted, so core e gets the
    contiguous slice x[off_e : off_e + gs_e], padded to a common `cap`.
  - LoRA routing trick: with A=8 adapters and rank R=16, the per-expert
    concatenation A_cat = [A[0,e] .. A[7,e]] is [1024, 128]. Compute
    inter_all = x @ A_cat densely for ALL adapters, then multiply by a
    per-token mask M[j, t] = s_{a_t} * (j in adapter-a_t block) and feed
    the masked inter into B_cat = [B[0,e]; ..; B[7,e]] ([128, 1024]).
    This turns the ragged adapter grouping into two dense matmuls and
    one elementwise mask — no on-device sorting or control flow.
  - The B-side matmul accumulates into the same PSUM tile as the base
    matmul, so base + lora is free.
  - All matmul operands are cast to bf16 on the host (fp32 PSUM
    accumulation on the PE); output is fp32.

Timeline engineering (the steady state is at the PE roofline; the wins
are all in the lead-in and the tail):
  - The runtime NEFF prelude (engine barrier + base-addr TENSOR_LOADs)
    ends ~5.2us; nothing bass-side runs earlier.
  - Warm-up matmuls have NO data dependency (they read an uninitialized
    SBUF tile) so the PE starts ramping the HAM clock immediately at
    ~6us instead of waiting for the first DMA to land (~8.5us). The HAM
    grants full clock after ~3.5us of sustained PE activity; a multi-us
    PE idle during/after the ramp trips a 7-14us half-duty penalty
    window, so the warm-up count is sized to bridge until the first
    real operands (a8 + xt8 g0) have landed.
  - DMA enqueues are split across BOTH hardware-DGE engines (Sync and
    Activation) so the ~0.65us/enqueue serialization halves, and each
    queue's FIFO is ordered by first-use time. The scalar engine issues
    no activation ops (all PSUM->SBUF copies are on vector), so it
    never loads the activation table and can enqueue from ~6us.
  - First group runs k-major across THREE token tiles so one W k-chunk
    feeds 6 matmuls (~1.3us) — matching the per-queue W delivery rate
    while the other queue streams x/mask/bcat concurrently.
  - The bulk x groups are dep-chained behind most of W (a dummy vector
    op reading W k5 + the head of the not-yet-loaded xt region gives
    the bulk DMAs a WAR dependency) so they don't steal bandwidth from
    the critical path, and are enqueued per-group in consumption order.
  - Output DMAs ride the scalar queue; the last groups are split so the
    final transfer is one 128-token tile, keeping the tail short.
"""

import numpy as np

T, E, IN, OUT, A, R = 16384, 8, 1024, 1024, 8, 16
NCORES = 8
AR = A * R  # 128
KC = IN // 128  # 8 contraction chunks
OC = OUT // 512  # 2 output column chunks
WARMUP = 17  # see timeline notes above

_compiled_cache: dict[int, object] = {}


# ---------------------------------------------------------------------------
# walrus in this container accepts at most 1 sync-wait command per
# instruction; Tile attaches more. Split excess waits onto no-ops.
# ---------------------------------------------------------------------------


def _apply_tile_wait_patch():
    import bass_rust
    import concourse.tile as tile
    from concourse import mybir
    from concourse.vector_clock import ScopedClock

    if getattr(tile.TileContext, "_wait_split_patched", False):
        return

    MAX_WAITS = 1

    def _split_excess_waits(nc):
        for fn in nc.m.functions:
            for blk in fn.blocks:
                insts = blk.instructions  # live list
                i = 0
                while i < len(insts):
                    inst = insts[i]
                    si = inst.sync_info
                    if si is not None and len(si.on_wait) > MAX_WAITS:
                        waits = list(si.on_wait)
                        keep = waits[-MAX_WAITS:]
                        excess = waits[:-MAX_WAITS]
                        inst.sync_info = bass_rust.SyncInfo(
                            on_wait=keep, on_update=list(si.on_update)
                        )
                        pos = i
                        for k in range(0, len(excess), MAX_WAITS):
                            nop = mybir.InstNoOp(
                                name=f"{inst.name}-hoistw{k}",
                                engine=inst.engine,
                                bass_nofuse=True,
                                sync_info=mybir.SyncInfo(
                                    on_wait=excess[k : k + MAX_WAITS], on_update=[]
                                ),
                            )
                            insts.insert(pos, nop)
                            pos += 1
                            i += 1
                    i += 1

    def _split_drain_and_barrier(self, tick_clock, wait_clock):
        nc = self.nc
        drain_inst = nc.sync.drain()
        wait_clock.add_sem_waits(
            drain_inst.ins, ScopedClock({None: tick_clock.global_clock})
        )
        si = drain_inst.ins.sync_info
        if si is not None and len(si.on_wait) > MAX_WAITS:
            waits = list(si.on_wait)
            drain_inst.ins.sync_info = bass_rust.SyncInfo(
                on_wait=waits[:MAX_WAITS], on_update=list(si.on_update)
            )
            for k in range(MAX_WAITS, len(waits), MAX_WAITS):
                extra = nc.sync.drain()
                extra.ins.sync_info = bass_rust.SyncInfo(
                    on_wait=waits[k : k + MAX_WAITS], on_update=[]
                )

        import os as _os

        # Gather/release barrier: 2 sem hops (~1us) instead of the
        # 5-hop chained form (~2.5us).
        nc.all_engine_barrier(sem_only=True)
        assert self.sems is not None
        popped = nc._tile_sem_poison_stack.pop()
        assert popped is self._sem_poison
        nc.clear_and_free_semaphores(list(self.sems.allocated().values()))
        if _os.environ.get("LORA_LEAN_TAIL", "1") != "1":
            # Second barrier only matters for kernels that continue past
            # the TileContext; ours ends here (sem clears trail on gpsimd).
            nc.all_engine_barrier()

        _split_excess_waits(nc)

    tile.TileContext._drain_and_barrier = _split_drain_and_barrier
    tile.TileContext._wait_split_patched = True


# ---------------------------------------------------------------------------
# Bass program (one SPMD NeuronCore program, parameterized by cap)
# ---------------------------------------------------------------------------


def _build(cap: int):
    import concourse.bass as bass
    import concourse.tile as tile
    from concourse import mybir

    _apply_tile_wait_patch()

    ntt = cap // 128  # token tiles
    ngr = (cap + 511) // 512  # inter groups of up to 512 tokens

    bf16 = mybir.dt.bfloat16
    f32 = mybir.dt.float32
    f8e4 = mybir.dt.float8e4
    DR = mybir.MatmulPerfMode.DoubleRow

    # partition-id preamble and monotonic sems are unused here — skip them.
    # Also skip the single __init__ barrier (bass.py:7557): it only
    # orders the const-AP memsets, which nothing in this kernel reads.
    _orig_aeb = bass.Bass.all_engine_barrier
    bass.Bass.all_engine_barrier = lambda self, **kw: None
    try:
        nc = bass.Bass(enable_partition_id=False, monotonic_sem_count=0)
    finally:
        bass.Bass.all_engine_barrier = _orig_aeb
    # XT[g, k, p, c] = x_e[512g + c, 128k + p]
    XT = nc.dram_tensor("xt", [ngr, KC, 128, 512], bf16, kind="ExternalInput")
    XT8 = nc.dram_tensor("xt8", [ngr, KC, 128, 512], f8e4, kind="ExternalInput")
    W = nc.dram_tensor("w", [KC, 128, OUT], bf16, kind="ExternalInput")
    A8 = nc.dram_tensor("a8", [128, KC, AR], f8e4, kind="ExternalInput")
    BCAT = nc.dram_tensor("bcat", [AR, OUT], bf16, kind="ExternalInput")
    MASKT = nc.dram_tensor("maskt", [AR, cap], bf16, kind="ExternalInput")
    OUTD = nc.dram_tensor("out", [cap, OUT], bf16, kind="ExternalOutput")

    def gslice(g):
        t0 = g * 512
        return t0, min(512, cap - t0)

    # tiles in group 0 processed k-major in one fused block
    fancy = cap >= 512  # group 0 has 4 full tiles
    nfuse = 2

    with tile.TileContext(nc) as tc:
        with (
            tc.tile_pool(name="big", bufs=1) as big,
            tc.tile_pool(name="outp", bufs=2) as outp,
            tc.tile_pool(name="psi", bufs=2, space="PSUM") as psi,
            tc.tile_pool(name="pso", bufs=6, space="PSUM") as pso,
        ):
            # --- PE warm-up: no DMA dependency (reads mostly
            # uninitialized SBUF; the 1-column memset just allocates the
            # tile); output psum is a pso slot that later tiles reuse.
            warm_sb = big.tile([128, 4, AR], f8e4)
            nc.vector.memset(warm_sb[:, 0, 0:1], 0)
            wps = pso.tile([128, 512], f32, name="pswarm", tag="pso")
            for i in range(WARMUP):
                nc.tensor.matmul(
                    wps[:], warm_sb[:, 0, :], warm_sb[:],
                    start=(i == 0), stop=(i == WARMUP - 1),
                )

            # --- DMA enqueues. Two HWDGE queues; concurrent queues
            # fair-share HBM read bandwidth, a solo queue gets nearly all
            # of it. The critical chain (x tiles 0-2 + W, which feed the
            # fused k-major block) rides the sync queue ALONE; everything
            # else rides the scalar queue, dep-chained behind the x head
            # so it cannot steal bandwidth from the critical chain.
            # xt_sb[p, g, k, c] = bf16 x for the base-matmul stationary
            xt_sb = big.tile([128, ngr, KC, 512], bf16)
            c0 = min(nfuse * 128, cap) if fancy else min(512, cap)
            nc.sync.dma_start(
                xt_sb[:, 0, :, :c0],
                XT[0, :, :, :c0].rearrange("k p c -> p k c"),
            )
            w_sb = big.tile([128, KC, OUT], bf16)
            for k in range(KC):
                nc.sync.dma_start(
                    w_sb[:, k : k + 1, :],
                    W[k : k + 1, :, :].rearrange("k p c -> p k c"),
                )

            a8_sb = big.tile([128, KC, AR], f8e4)
            xt8_sb = big.tile([128, ngr, KC, 512], f8e4)
            if fancy:
                # Gate the scalar queue behind the x head: dummy vector op
                # reading the landed xth head and the not-yet-written a8
                # region gives a8's DMA (and everything FIFO-behind it) a
                # WAR dependency on the critical x transfer.
                dep2_sb = big.tile([128, 1], f32)
                nc.vector.scalar_tensor_tensor(
                    dep2_sb[:],
                    w_sb[:, 0, OUT - 1 : OUT],
                    1.0,
                    a8_sb[:, 0, 0:1],
                    mybir.AluOpType.mult,
                    mybir.AluOpType.mult,
                )
            nc.scalar.dma_start(a8_sb[:], A8[:])
            # xt8_sb[p, g, k, c] = fp8(x_e[512g + c, 128k + p])
            nc.scalar.dma_start(
                xt8_sb[:, 0, :, :], XT8[0, :, :, :].rearrange("k p c -> p k c")
            )
            maskt_sb = big.tile([AR, cap], bf16)
            m0 = min(512, cap)
            nc.scalar.dma_start(maskt_sb[:, :m0], MASKT[:, :m0])
            if cap > c0:
                g0w = min(512, cap)
                nc.scalar.dma_start(
                    xt_sb[:, 0, :, c0:g0w],
                    XT[0, :, :, c0:g0w].rearrange("k p c -> p k c"),
                )
            b_sb = big.tile([AR, OUT], bf16)
            nc.scalar.dma_start(b_sb[:], BCAT[:])
            if cap > m0:
                nc.scalar.dma_start(maskt_sb[:, m0:], MASKT[:, m0:])

            if ngr > 1:
                # Dummy op READING W k5 and the head of the not-yet-loaded
                # xt region: the bulk x DMAs then carry a WAR dependency on
                # it, so they wait for most of W before competing for HBM
                # bandwidth. (A write INTO xt would race the DMA.)
                dep_sb = big.tile([128, 1], f32)
                nc.vector.scalar_tensor_tensor(
                    dep_sb[:],
                    w_sb[:, KC - 3, OUT - 1 : OUT],
                    1.0,
                    xt_sb[:, 1, 0, 0:1],
                    mybir.AluOpType.mult,
                    mybir.AluOpType.mult,
                )
                # bulk groups, per-group DMAs in consumption order
                for g in range(1, ngr):
                    nc.sync.dma_start(
                        xt8_sb[:, g, :, :],
                        XT8[g, :, :, :].rearrange("k p c -> p k c"),
                    )
                    nc.sync.dma_start(
                        xt_sb[:, g, :, :],
                        XT[g, :, :, :].rearrange("k p c -> p k c"),
                    )

            interm_sb = big.tile([AR, cap], bf16)

            def phase1(g):
                # inter_all = (x8 @ (64*A_cat))^T via fp8 DoubleRow (two
                # 128-deep k-chunks per instruction), masked by s_a/64
                # -> interm_sb
                t0, wg = gslice(g)
                ps = psi.tile([128, 512], f32, name=f"psi{g}", tag="psi")
                for i in range(KC // 2):
                    nc.tensor.matmul(
                        ps[:, :wg],
                        a8_sb[:, 2 * i : 2 * i + 2, :],
                        xt8_sb[:, g, 2 * i : 2 * i + 2, :wg],
                        start=(i == 0),
                        stop=(i == KC // 2 - 1),
                        perf_mode=DR,
                    )
                nc.vector.scalar_tensor_tensor(
                    interm_sb[:, t0 : t0 + wg],
                    ps[:, :wg],
                    1.0,
                    maskt_sb[:, t0 : t0 + wg],
                    mybir.AluOpType.mult,
                    mybir.AluOpType.mult,
                )

            def copies(tt, pss, o_sb):
                # psum -> output stage, oc0 on vector, oc1 on scalar
                j = tt % 4
                for oc in range(OC):
                    dst = o_sb[:, j * OUT + oc * 512 : j * OUT + oc * 512 + 512]
                    if oc == 0:
                        nc.vector.tensor_copy(dst, pss[oc][:])
                    else:
                        nc.scalar.copy(dst, pss[oc][:])

            def bside(tt, pss):
                ts0 = tt * 128
                for oc in range(OC):
                    nc.tensor.matmul(
                        pss[oc][:],
                        interm_sb[:, ts0 : ts0 + 128],
                        b_sb[:, oc * 512 : oc * 512 + 512],
                        start=False,
                        stop=True,
                    )

            def token_tile(tt, o_sb):
                # base k-loop into 2 psum banks (one per 512-wide output
                # chunk), + 1 lora matmul each, then copy to group stage.
                g, j = tt // 4, tt % 4
                pss = [
                    pso.tile([128, 512], f32, name=f"psod{tt}_{i}", tag="pso")
                    for i in range(OC)
                ]
                for k in range(KC):
                    for oc in range(OC):
                        nc.tensor.matmul(
                            pss[oc][:],
                            xt_sb[:, g, k, j * 128 : j * 128 + 128],
                            w_sb[:, k, oc * 512 : oc * 512 + 512],
                            start=(k == 0),
                            stop=False,
                        )
                bside(tt, pss)
                copies(tt, pss, o_sb)

            def out_dma(g, o_sb):
                t0, wg = gslice(g)
                ntg = wg // 128
                if g < ngr - 2:
                    # one output DMA per group: [128, ntg, OUT] rows
                    nc.scalar.dma_start(
                        OUTD[t0 : t0 + wg, :].rearrange(
                            "(j p) c -> p j c", p=128
                        ),
                        o_sb[:].rearrange("p (j c) -> p j c", c=OUT),
                    )
                elif g == ngr - 2 and ntg > 1:
                    # split the second-to-last group so its output mostly
                    # clears the wire before the final tile's transfer
                    h = ntg // 2
                    nc.scalar.dma_start(
                        OUTD[t0 : t0 + h * 128, :].rearrange(
                            "(j p) c -> p j c", p=128
                        ),
                        o_sb[:, : h * OUT].rearrange("p (j c) -> p j c", c=OUT),
                    )
                    nc.scalar.dma_start(
                        OUTD[t0 + h * 128 : t0 + wg, :].rearrange(
                            "(j p) c -> p j c", p=128
                        ),
                        o_sb[:, h * OUT :].rearrange("p (j c) -> p j c", c=OUT),
                    )
                else:
                    # last group: per-(tile, oc) DMAs so the final
                    # transfer starts right after its own copy instead of
                    # waiting for all of the group's copies.
                    for j in range(wg // 128):
                        for oc in range(OC):
                            nc.scalar.dma_start(
                                OUTD[
                                    t0 + j * 128 : t0 + j * 128 + 128,
                                    oc * 512 : oc * 512 + 512,
                                ],
                                o_sb[
                                    :,
                                    j * OUT + oc * 512 : j * OUT + oc * 512 + 512,
                                ],
                            )

            if fancy:
                # ---- group 0: fused k-major over tiles 0-1 (their x head
                # is the 0.5MB critical transfer the warm-up bridges to),
                # with phase1(0) and tile 2's base k-loop woven in so the
                # PE never waits on the (late, scalar-queue) A-side
                # operands or the vector mask STT.
                o_sb = outp.tile([128, 4 * OUT], bf16, name="og0", tag="outp")
                pss = [
                    pso.tile([128, 512], f32, name=f"psod{t}_{i}", tag="pso")
                    for t in range(nfuse)
                    for i in range(OC)
                ]
                for k in range(KC):
                    for t in range(nfuse):
                        for oc in range(OC):
                            nc.tensor.matmul(
                                pss[t * OC + oc][:],
                                xt_sb[:, 0, k, t * 128 : t * 128 + 128],
                                w_sb[:, k, oc * 512 : oc * 512 + 512],
                                start=(k == 0),
                                stop=False,
                            )
                phase1(0)
                # tile 2 base between phase1(0) and the B-sides: 3.4us of
                # dependency-free PE work while the mask STT completes.
                ps2 = [
                    pso.tile([128, 512], f32, name=f"psod2_{i}", tag="pso")
                    for i in range(OC)
                ]
                for k in range(KC):
                    for oc in range(OC):
                        nc.tensor.matmul(
                            ps2[oc][:],
                            xt_sb[:, 0, k, 256:384],
                            w_sb[:, k, oc * 512 : oc * 512 + 512],
                            start=(k == 0),
                            stop=False,
                        )
                bside(0, pss[0:2])
                copies(0, pss[0:2], o_sb)
                bside(1, pss[2:4])
                copies(1, pss[2:4], o_sb)
                bside(2, ps2)
                copies(2, ps2, o_sb)
                token_tile(3, o_sb)
                if ngr > 1:
                    phase1(1)
                out_dma(0, o_sb)
                g_start = 1
            else:
                phase1(0)
                g_start = 0

            for g in range(g_start, ngr):
                t0, wg = gslice(g)
                ntg = wg // 128
                # group output stage: o_sb[p, j*OUT + c] = out[t0+128j+p, c]
                o_sb = outp.tile(
                    [128, ntg * OUT], bf16, name=f"og{g}", tag="outp"
                )
                tts = list(range(t0 // 128, (t0 + wg) // 128))
                for j, tt in enumerate(tts):
                    token_tile(tt, o_sb)
                    # emit next group's phase 1 after this group's LAST
                    # token tile: by then its (dep-chained, late) x8 chunk
                    # has arrived, and the mask STT still completes before
                    # the first B-side matmul of group g+1 needs it.
                    if j == len(tts) - 1 and g + 1 < ngr:
                        phase1(g + 1)
                out_dma(g, o_sb)

    return nc


def _get_compiled(cap: int):
    if cap not in _compiled_cache:
        _compiled_cache[cap] = _build(cap)
    return _compiled_cache[cap]


# ---------------------------------------------------------------------------
# Host-side routing + execution
# ---------------------------------------------------------------------------


def _reference_numpy(x, group_sizes, adapter_indices_sorted, weight, lora_A, lora_B, lora_scaling):
    """Fallback replicating the jax reference exactly (only used for
    degenerate group_sizes that do not sum to T)."""
    x = np.asarray(x, np.float32)
    gs = np.asarray(group_sizes, np.int64)
    adapter = np.asarray(adapter_indices_sorted, np.int64)
    out = np.zeros((x.shape[0], weight.shape[2]), np.float32)
    # base: ragged_dot semantics (groups from cumsum, tail rows -> 0)
    offs = np.minimum(np.concatenate([[0], np.cumsum(gs)]), x.shape[0])
    for e in range(E):
        s, t = offs[e], offs[e + 1]
        if t > s:
            out[s:t] = x[s:t] @ weight[e]
    # lora: expert ids via repeat padded with the final value
    rep = np.repeat(np.arange(E), np.maximum(gs, 0))[: x.shape[0]]
    if rep.size == 0:
        rep = np.zeros(x.shape[0], np.int64)
    elif rep.size < x.shape[0]:
        rep = np.concatenate(
            [rep, np.full(x.shape[0] - rep.size, rep[-1], np.int64)]
        )
    for t in range(x.shape[0]):
        e, a = rep[t], adapter[t]
        inter = x[t] @ lora_A[a, e]
        out[t] += lora_scaling[a] * (inter @ lora_B[a, e])
    return out


def kernel(x, group_sizes, adapter_indices_sorted, weight, lora_A, lora_B, lora_scaling):
    import ml_dtypes

    x = np.ascontiguousarray(np.asarray(x, np.float32))
    weight = np.asarray(weight, np.float32)
    lora_A = np.asarray(lora_A, np.float32)
    lora_B = np.asarray(lora_B, np.float32)
    scaling = np.asarray(lora_scaling, np.float32)
    gs = np.asarray(group_sizes).astype(np.int64)
    adapter = np.asarray(adapter_indices_sorted).astype(np.int64)

    if gs.sum() != T or (gs < 0).any():
        return _reference_numpy(
            x, gs, adapter, weight, lora_A, lora_B, scaling
        )

    from concourse.bass_utils import run_bass_kernel_spmd

    bf = ml_dtypes.bfloat16
    f8 = ml_dtypes.float8_e4m3
    cap = int(max(128, -(-int(gs.max()) // 128) * 128))
    nc = _get_compiled(cap)

    offs = np.concatenate([[0], np.cumsum(gs)])
    in_maps = []
    for e in range(NCORES):
        n = int(gs[e])
        s = int(offs[e])
        ngr = (cap + 511) // 512
        xe = np.zeros((ngr * 512, IN), np.float32)
        xe[:n] = x[s : s + n]
        # [ngr, KC, 128, 512]: XT[g, k, p, t] = x_e[512g+t, 128k+p]
        xe_t = xe.T.reshape(KC, 128, ngr, 512).transpose(2, 0, 1, 3)
        xt = np.ascontiguousarray(xe_t.astype(bf))
        xt8 = np.ascontiguousarray(xe_t.astype(f8))
        w = np.ascontiguousarray(weight[e].reshape(KC, 128, OUT).astype(bf))
        # A_cat[:, a*R+r] = lora_A[a, e, :, r] -> [128, KC, AR], scaled by
        # 64 to land in fp8 e4m3's normal range; the mask divides it out.
        acat_full = lora_A[:, e].transpose(1, 0, 2).reshape(IN, AR)
        a8 = np.ascontiguousarray(
            (acat_full.reshape(KC, 128, AR) * 64.0).transpose(1, 0, 2).astype(f8)
        )
        bcat = np.ascontiguousarray(lora_B[:, e].reshape(AR, OUT).astype(bf))
        ae = adapter[s : s + n]
        m = np.zeros((A, cap), np.float32)
        m[ae, np.arange(n)] = scaling[ae] / 64.0
        maskt = np.ascontiguousarray(np.repeat(m, R, axis=0).astype(bf))
        in_maps.append(
            {"xt": xt, "xt8": xt8, "w": w, "a8": a8, "bcat": bcat, "maskt": maskt}
        )

    res = run_bass_kernel_spmd(nc, in_maps, list(range(NCORES)))

    out = np.empty((T, OUT), np.float32)
    for e in range(NCORES):
        n = int(gs[e])
        if n:
            out[int(offs[e]) : int(offs[e]) + n] = (
                res.results[e]["out"][:n].astype(np.float32)
            )
    return out


# revision 15
# speedup vs baseline: 1.0576x; 1.0576x over previous
"""Trainium2 Bass kernel for nn_LoRAExpert (moe_routing).

Per token t (expert e_t from contiguous group_sizes, adapter a_t):

    out[t] = x[t] @ W[e_t] + s_{a_t} * (x[t] @ A[a_t, e_t]) @ B[a_t, e_t]

Strategy (expert-parallel over 8 NeuronCores):
  - Host routes tokens: x is already expert-sorted, so core e gets the
    contiguous slice x[off_e : off_e + gs_e], padded to a common `cap`.
  - LoRA routing trick: with A=8 adapters and rank R=16, the per-expert
    concatenation A_cat = [A[0,e] .. A[7,e]] is [1024, 128]. Compute
    inter_all = x @ A_cat densely for ALL adapters, then multiply by a
    per-token mask M[j, t] = s_{a_t} * (j in adapter-a_t block) and feed
    the masked inter into B_cat = [B[0,e]; ..; B[7,e]] ([128, 1024]).
    This turns the ragged adapter grouping into two dense matmuls and
    one elementwise mask — no on-device sorting or control flow.
  - The B-side matmul accumulates into the same PSUM tile as the base
    matmul, so base + lora is free.
  - All matmul operands are cast to bf16 on the host (fp32 PSUM
    accumulation on the PE); output is fp32.

Timeline engineering (the steady state is at the PE roofline; the wins
are all in the lead-in and the tail):
  - The runtime NEFF prelude (engine barrier + base-addr TENSOR_LOADs)
    ends ~5.2us; nothing bass-side runs earlier.
  - Warm-up matmuls have NO data dependency (they read an uninitialized
    SBUF tile) so the PE starts ramping the HAM clock immediately at
    ~6us instead of waiting for the first DMA to land (~8.5us). The HAM
    grants full clock after ~3.5us of sustained PE activity; a multi-us
    PE idle during/after the ramp trips a 7-14us half-duty penalty
    window, so the warm-up count is sized to bridge until the first
    real operands (a8 + xt8 g0) have landed.
  - DMA enqueues are split across BOTH hardware-DGE engines (Sync and
    Activation) so the ~0.65us/enqueue serialization halves, and each
    queue's FIFO is ordered by first-use time. The scalar engine issues
    no activation ops (all PSUM->SBUF copies are on vector), so it
    never loads the activation table and can enqueue from ~6us.
  - First group runs k-major across THREE token tiles so one W k-chunk
    feeds 6 matmuls (~1.3us) — matching the per-queue W delivery rate
    while the other queue streams x/mask/bcat concurrently.
  - The bulk x groups are dep-chained behind most of W (a dummy vector
    op reading W k5 + the head of the not-yet-loaded xt region gives
    the bulk DMAs a WAR dependency) so they don't steal bandwidth from
    the critical path, and are enqueued per-group in consumption order.
  - Output DMAs ride the scalar queue; the last groups are split so the
    final transfer is one 128-token tile, keeping the tail short.
"""

import numpy as np

T, E, IN, OUT, A, R = 16384, 8, 1024, 1024, 8, 16
NCORES = 8
AR = A * R  # 128
KC = IN // 128  # 8 contraction chunks
OC = OUT // 512  # 2 output column chunks
WARMUP = 6  # free-running warm-ups before the self-clocked phase B

_compiled_cache: dict[int, object] = {}


# ---------------------------------------------------------------------------
# walrus in this container accepts at most 1 sync-wait command per
# instruction; Tile attaches more. Split excess waits onto no-ops.
# ---------------------------------------------------------------------------


def _apply_tile_wait_patch():
    import bass_rust
    import concourse.tile as tile
    from concourse import mybir
    from concourse.vector_clock import ScopedClock

    if getattr(tile.TileContext, "_wait_split_patched", False):
        return

    MAX_WAITS = 1

    def _split_excess_waits(nc):
        for fn in nc.m.functions:
            for blk in fn.blocks:
                insts = blk.instructions  # live list
                i = 0
                while i < len(insts):
                    inst = insts[i]
                    si = inst.sync_info
                    if si is not None and len(si.on_wait) > MAX_WAITS:
                        waits = list(si.on_wait)
                        keep = waits[-MAX_WAITS:]
                        excess = waits[:-MAX_WAITS]
                        inst.sync_info = bass_rust.SyncInfo(
                            on_wait=keep, on_update=list(si.on_update)
                        )
                        pos = i
                        for k in range(0, len(excess), MAX_WAITS):
                            nop = mybir.InstNoOp(
                                name=f"{inst.name}-hoistw{k}",
                                engine=inst.engine,
                                bass_nofuse=True,
                                sync_info=mybir.SyncInfo(
                                    on_wait=excess[k : k + MAX_WAITS], on_update=[]
                                ),
                            )
                            insts.insert(pos, nop)
                            pos += 1
                            i += 1
                    i += 1

    def _split_drain_and_barrier(self, tick_clock, wait_clock):
        nc = self.nc
        drain_inst = nc.sync.drain()
        wait_clock.add_sem_waits(
            drain_inst.ins, ScopedClock({None: tick_clock.global_clock})
        )
        si = drain_inst.ins.sync_info
        if si is not None and len(si.on_wait) > MAX_WAITS:
            waits = list(si.on_wait)
            drain_inst.ins.sync_info = bass_rust.SyncInfo(
                on_wait=waits[:MAX_WAITS], on_update=list(si.on_update)
            )
            for k in range(MAX_WAITS, len(waits), MAX_WAITS):
                extra = nc.sync.drain()
                extra.ins.sync_info = bass_rust.SyncInfo(
                    on_wait=waits[k : k + MAX_WAITS], on_update=[]
                )

        import os as _os

        # Gather/release barrier: 2 sem hops (~1us) instead of the
        # 5-hop chained form (~2.5us).
        nc.all_engine_barrier(sem_only=True)
        assert self.sems is not None
        popped = nc._tile_sem_poison_stack.pop()
        assert popped is self._sem_poison
        nc.clear_and_free_semaphores(list(self.sems.allocated().values()))
        if _os.environ.get("LORA_LEAN_TAIL", "1") != "1":
            # Second barrier only matters for kernels that continue past
            # the TileContext; ours ends here (sem clears trail on gpsimd).
            nc.all_engine_barrier()

        _split_excess_waits(nc)

    tile.TileContext._drain_and_barrier = _split_drain_and_barrier
    tile.TileContext._wait_split_patched = True


# ---------------------------------------------------------------------------
# Bass program (one SPMD NeuronCore program, parameterized by cap)
# ---------------------------------------------------------------------------


def _build(cap: int):
    import concourse.bass as bass
    import concourse.tile as tile
    from concourse import mybir

    _apply_tile_wait_patch()

    ntt = cap // 128  # token tiles
    ngr = (cap + 511) // 512  # inter groups of up to 512 tokens

    bf16 = mybir.dt.bfloat16
    f32 = mybir.dt.float32
    f8e4 = mybir.dt.float8e4
    DR = mybir.MatmulPerfMode.DoubleRow

    # partition-id preamble and monotonic sems are unused here — skip them.
    # Also skip the single __init__ barrier (bass.py:7557): it only
    # orders the const-AP memsets, which nothing in this kernel reads.
    _orig_aeb = bass.Bass.all_engine_barrier
    bass.Bass.all_engine_barrier = lambda self, **kw: None
    try:
        nc = bass.Bass(enable_partition_id=False, monotonic_sem_count=0)
    finally:
        bass.Bass.all_engine_barrier = _orig_aeb
    # XT[g, k, p, c] = x_e[512g + c, 128k + p]
    XT = nc.dram_tensor("xt", [ngr, KC, 128, 512], bf16, kind="ExternalInput")
    XT8 = nc.dram_tensor("xt8", [ngr, KC, 128, 512], f8e4, kind="ExternalInput")
    W = nc.dram_tensor("w", [KC, 128, OUT], bf16, kind="ExternalInput")
    A8 = nc.dram_tensor("a8", [128, KC, AR], f8e4, kind="ExternalInput")
    BCAT = nc.dram_tensor("bcat", [AR, OUT], bf16, kind="ExternalInput")
    MASKT = nc.dram_tensor("maskt", [AR, cap], bf16, kind="ExternalInput")
    OUTD = nc.dram_tensor("out", [cap, OUT], bf16, kind="ExternalOutput")

    def gslice(g):
        t0 = g * 512
        return t0, min(512, cap - t0)

    # tiles in group 0 processed k-major in one fused block
    fancy = cap >= 512  # group 0 has 4 full tiles
    nfuse = 2

    with tile.TileContext(nc) as tc:
        with (
            tc.tile_pool(name="big", bufs=1) as big,
            tc.tile_pool(name="outp", bufs=2) as outp,
            tc.tile_pool(name="psi", bufs=2, space="PSUM") as psi,
            tc.tile_pool(name="pso", bufs=6, space="PSUM") as pso,
        ):
            # --- PE warm-up, phase A: no DMA dependency (reads mostly
            # uninitialized SBUF; the 1-column memset just allocates the
            # tile); output psum is a pso slot that later tiles reuse.
            warm_sb = big.tile([128, 4, AR], bf16)
            nc.vector.memset(warm_sb[:, 0, 0:1], 0)
            wps = pso.tile([128, 512], f32, name="pswarm", tag="pso")
            for i in range(WARMUP):
                nc.tensor.matmul(
                    wps[:], warm_sb[:, 0, :], warm_sb[:],
                    start=(i == 0), stop=False,
                )

            # --- DMA enqueues. Two HWDGE queues; concurrent queues
            # fair-share HBM read bandwidth, a solo queue gets nearly all
            # of it. The critical chain (x tiles 0-1 in four column
            # chunks + W, which feed the fused k-major block) rides the
            # sync queue ALONE; everything else rides the scalar queue,
            # dep-chained behind W k0 so it cannot steal bandwidth from
            # the critical chain during the DMA-fabric ramp.
            # xt_sb[p, g, k, c] = bf16 x for the base-matmul stationary
            xt_sb = big.tile([128, ngr, KC, 512], bf16)
            c0 = min(nfuse * 128, cap) if fancy else min(512, cap)
            w_sb = big.tile([128, KC, OUT], bf16)
            if fancy:
                # interleave x chunks and the first W chunk in FIFO order
                # so the self-clocked warm-up (phase B below) tracks them
                csz = c0 // 4
                chunks = [(i * csz, (i + 1) * csz) for i in range(4)]
                for lo, hi in chunks[:2]:
                    nc.sync.dma_start(
                        xt_sb[:, 0, :, lo:hi],
                        XT[0, :, :, lo:hi].rearrange("k p c -> p k c"),
                    )
                nc.sync.dma_start(
                    w_sb[:, 0:1, :], W[0:1, :, :].rearrange("k p c -> p k c")
                )
                for lo, hi in chunks[2:]:
                    nc.sync.dma_start(
                        xt_sb[:, 0, :, lo:hi],
                        XT[0, :, :, lo:hi].rearrange("k p c -> p k c"),
                    )
                for k in range(1, KC):
                    nc.sync.dma_start(
                        w_sb[:, k : k + 1, :],
                        W[k : k + 1, :, :].rearrange("k p c -> p k c"),
                    )
                # --- PE warm-up, phase B (self-clocked): a tiny vector
                # copy re-marks the warm tile as each x chunk lands, and
                # each warm-up burst reads the mark, so the warm-up stream
                # ends when (and only when) the fused block's operands are
                # resident — no PE idle gap at ANY core clock / DMA ramp
                # combination, and no compile-time guess of arrival time.
                for ci, (lo, hi) in enumerate(chunks):
                    nc.vector.tensor_copy(
                        warm_sb[:, 1, ci : ci + 1], xt_sb[:, 0, 0, lo : lo + 1]
                    )
                    nburst = 2 if ci < 3 else 1
                    for i in range(nburst):
                        last = ci == 3 and i == nburst - 1
                        nc.tensor.matmul(
                            wps[:],
                            warm_sb[:, 0, :],
                            warm_sb[:],
                            start=False,
                            stop=last,
                        )
            else:
                nc.sync.dma_start(
                    xt_sb[:, 0, :, :c0],
                    XT[0, :, :, :c0].rearrange("k p c -> p k c"),
                )
                for k in range(KC):
                    nc.sync.dma_start(
                        w_sb[:, k : k + 1, :],
                        W[k : k + 1, :, :].rearrange("k p c -> p k c"),
                    )
                nc.tensor.matmul(
                    wps[:], warm_sb[:, 0, :], warm_sb[:],
                    start=False, stop=True,
                )

            a8_sb = big.tile([128, KC, AR], f8e4)
            xt8_sb = big.tile([128, ngr, KC, 512], f8e4)
            if fancy:
                # Gate the scalar queue behind the x head: dummy vector op
                # reading the landed xth head and the not-yet-written a8
                # region gives a8's DMA (and everything FIFO-behind it) a
                # WAR dependency on the critical x transfer.
                dep2_sb = big.tile([128, 1], f32)
                nc.vector.scalar_tensor_tensor(
                    dep2_sb[:],
                    w_sb[:, 0, OUT - 1 : OUT],
                    1.0,
                    a8_sb[:, 0, 0:1],
                    mybir.AluOpType.mult,
                    mybir.AluOpType.mult,
                )
            nc.scalar.dma_start(a8_sb[:], A8[:])
            # xt8_sb[p, g, k, c] = fp8(x_e[512g + c, 128k + p])
            nc.scalar.dma_start(
                xt8_sb[:, 0, :, :], XT8[0, :, :, :].rearrange("k p c -> p k c")
            )
            maskt_sb = big.tile([AR, cap], bf16)
            m0 = min(512, cap)
            if cap > c0:
                g0w = min(512, cap)
                nc.scalar.dma_start(
                    xt_sb[:, 0, :, c0:g0w],
                    XT[0, :, :, c0:g0w].rearrange("k p c -> p k c"),
                )
            nc.scalar.dma_start(maskt_sb[:, :m0], MASKT[:, :m0])
            b_sb = big.tile([AR, OUT], bf16)
            nc.scalar.dma_start(b_sb[:], BCAT[:])
            if cap > m0:
                nc.scalar.dma_start(maskt_sb[:, m0:], MASKT[:, m0:])

            if ngr > 1:
                # Dummy op READING W k5 and the head of the not-yet-loaded
                # xt region: the bulk x DMAs then carry a WAR dependency on
                # it, so they wait for most of W before competing for HBM
                # bandwidth. (A write INTO xt would race the DMA.)
                dep_sb = big.tile([128, 1], f32)
                nc.vector.scalar_tensor_tensor(
                    dep_sb[:],
                    w_sb[:, KC - 3, OUT - 1 : OUT],
                    1.0,
                    xt_sb[:, 1, 0, 0:1],
                    mybir.AluOpType.mult,
                    mybir.AluOpType.mult,
                )
                # bulk groups, per-group DMAs in consumption order
                for g in range(1, ngr):
                    nc.sync.dma_start(
                        xt8_sb[:, g, :, :],
                        XT8[g, :, :, :].rearrange("k p c -> p k c"),
                    )
                    nc.sync.dma_start(
                        xt_sb[:, g, :, :],
                        XT[g, :, :, :].rearrange("k p c -> p k c"),
                    )

            interm_sb = big.tile([AR, cap], bf16)

            def phase1(g):
                # inter_all = (x8 @ (64*A_cat))^T via fp8 DoubleRow (two
                # 128-deep k-chunks per instruction), masked by s_a/64
                # -> interm_sb
                t0, wg = gslice(g)
                ps = psi.tile([128, 512], f32, name=f"psi{g}", tag="psi")
                for i in range(KC // 2):
                    nc.tensor.matmul(
                        ps[:, :wg],
                        a8_sb[:, 2 * i : 2 * i + 2, :],
                        xt8_sb[:, g, 2 * i : 2 * i + 2, :wg],
                        start=(i == 0),
                        stop=(i == KC // 2 - 1),
                        perf_mode=DR,
                    )
                nc.vector.scalar_tensor_tensor(
                    interm_sb[:, t0 : t0 + wg],
                    ps[:, :wg],
                    1.0,
                    maskt_sb[:, t0 : t0 + wg],
                    mybir.AluOpType.mult,
                    mybir.AluOpType.mult,
                )

            def copies(tt, pss, o_sb):
                # psum -> output stage, oc0 on vector, oc1 on scalar
                j = tt % 4
                for oc in range(OC):
                    dst = o_sb[:, j * OUT + oc * 512 : j * OUT + oc * 512 + 512]
                    if oc == 0:
                        nc.vector.tensor_copy(dst, pss[oc][:])
                    else:
                        nc.scalar.copy(dst, pss[oc][:])

            def bside(tt, pss):
                ts0 = tt * 128
                for oc in range(OC):
                    nc.tensor.matmul(
                        pss[oc][:],
                        interm_sb[:, ts0 : ts0 + 128],
                        b_sb[:, oc * 512 : oc * 512 + 512],
                        start=False,
                        stop=True,
                    )

            def token_tile(tt, o_sb):
                # base k-loop into 2 psum banks (one per 512-wide output
                # chunk), + 1 lora matmul each, then copy to group stage.
                g, j = tt // 4, tt % 4
                pss = [
                    pso.tile([128, 512], f32, name=f"psod{tt}_{i}", tag="pso")
                    for i in range(OC)
                ]
                for k in range(KC):
                    for oc in range(OC):
                        nc.tensor.matmul(
                            pss[oc][:],
                            xt_sb[:, g, k, j * 128 : j * 128 + 128],
                            w_sb[:, k, oc * 512 : oc * 512 + 512],
                            start=(k == 0),
                            stop=False,
                        )
                bside(tt, pss)
                copies(tt, pss, o_sb)

            def out_dma(g, o_sb):
                t0, wg = gslice(g)
                ntg = wg // 128
                if g < ngr - 2:
                    # one output DMA per group: [128, ntg, OUT] rows
                    nc.scalar.dma_start(
                        OUTD[t0 : t0 + wg, :].rearrange(
                            "(j p) c -> p j c", p=128
                        ),
                        o_sb[:].rearrange("p (j c) -> p j c", c=OUT),
                    )
                elif g == ngr - 2 and ntg > 1:
                    # split the second-to-last group so its output mostly
                    # clears the wire before the final tile's transfer
                    h = ntg // 2
                    nc.scalar.dma_start(
                        OUTD[t0 : t0 + h * 128, :].rearrange(
                            "(j p) c -> p j c", p=128
                        ),
                        o_sb[:, : h * OUT].rearrange("p (j c) -> p j c", c=OUT),
                    )
                    nc.scalar.dma_start(
                        OUTD[t0 + h * 128 : t0 + wg, :].rearrange(
                            "(j p) c -> p j c", p=128
                        ),
                        o_sb[:, h * OUT :].rearrange("p (j c) -> p j c", c=OUT),
                    )
                else:
                    # last group: per-(tile, oc) DMAs so the final
                    # transfer starts right after its own copy instead of
                    # waiting for all of the group's copies.
                    for j in range(wg // 128):
                        for oc in range(OC):
                            nc.scalar.dma_start(
                                OUTD[
                                    t0 + j * 128 : t0 + j * 128 + 128,
                                    oc * 512 : oc * 512 + 512,
                                ],
                                o_sb[
                                    :,
                                    j * OUT + oc * 512 : j * OUT + oc * 512 + 512,
                                ],
                            )

            if fancy:
                # ---- group 0: fused k-major over tiles 0-1 (their x head
                # is the 0.5MB critical transfer the warm-up bridges to),
                # with phase1(0) and tile 2's base k-loop woven in so the
                # PE never waits on the (late, scalar-queue) A-side
                # operands or the vector mask STT.
                o_sb = outp.tile([128, 4 * OUT], bf16, name="og0", tag="outp")
                pss = [
                    pso.tile([128, 512], f32, name=f"psod{t}_{i}", tag="pso")
                    for t in range(nfuse)
                    for i in range(OC)
                ]
                for k in range(KC):
                    for t in range(nfuse):
                        for oc in range(OC):
                            nc.tensor.matmul(
                                pss[t * OC + oc][:],
                                xt_sb[:, 0, k, t * 128 : t * 128 + 128],
                                w_sb[:, k, oc * 512 : oc * 512 + 512],
                                start=(k == 0),
                                stop=False,
                            )
                phase1(0)
                # tile 2 base between phase1(0) and the B-sides: 3.4us of
                # dependency-free PE work while the mask STT completes.
                ps2 = [
                    pso.tile([128, 512], f32, name=f"psod2_{i}", tag="pso")
                    for i in range(OC)
                ]
                for k in range(KC):
                    for oc in range(OC):
                        nc.tensor.matmul(
                            ps2[oc][:],
                            xt_sb[:, 0, k, 256:384],
                            w_sb[:, k, oc * 512 : oc * 512 + 512],
                            start=(k == 0),
                            stop=False,
                        )
                bside(0, pss[0:2])
                copies(0, pss[0:2], o_sb)
                bside(1, pss[2:4])
                copies(1, pss[2:4], o_sb)
                bside(2, ps2)
                copies(2, ps2, o_sb)
                token_tile(3, o_sb)
                if ngr > 1:
                    phase1(1)
                out_dma(0, o_sb)
                g_start = 1
            else:
                phase1(0)
                g_start = 0

            for g in range(g_start, ngr):
                t0, wg = gslice(g)
                ntg = wg // 128
                # group output stage: o_sb[p, j*OUT + c] = out[t0+128j+p, c]
                o_sb = outp.tile(
                    [128, ntg * OUT], bf16, name=f"og{g}", tag="outp"
                )
                tts = list(range(t0 // 128, (t0 + wg) // 128))
                for j, tt in enumerate(tts):
                    token_tile(tt, o_sb)
                    # emit next group's phase 1 after this group's LAST
                    # token tile: by then its (dep-chained, late) x8 chunk
                    # has arrived, and the mask STT still completes before
                    # the first B-side matmul of group g+1 needs it.
                    if j == len(tts) - 1 and g + 1 < ngr:
                        phase1(g + 1)
                out_dma(g, o_sb)

    return nc


def _get_compiled(cap: int):
    if cap not in _compiled_cache:
        _compiled_cache[cap] = _build(cap)
    return _compiled_cache[cap]


# ---------------------------------------------------------------------------
# Host-side routing + execution
# ---------------------------------------------------------------------------


def _reference_numpy(x, group_sizes, adapter_indices_sorted, weight, lora_A, lora_B, lora_scaling):
    """Fallback replicating the jax reference exactly (only used for
    degenerate group_sizes that do not sum to T)."""
    x = np.asarray(x, np.float32)
    gs = np.asarray(group_sizes, np.int64)
    adapter = np.asarray(adapter_indices_sorted, np.int64)
    out = np.zeros((x.shape[0], weight.shape[2]), np.float32)
    # base: ragged_dot semantics (groups from cumsum, tail rows -> 0)
    offs = np.minimum(np.concatenate([[0], np.cumsum(gs)]), x.shape[0])
    for e in range(E):
        s, t = offs[e], offs[e + 1]
        if t > s:
            out[s:t] = x[s:t] @ weight[e]
    # lora: expert ids via repeat padded with the final value
    rep = np.repeat(np.arange(E), np.maximum(gs, 0))[: x.shape[0]]
    if rep.size == 0:
        rep = np.zeros(x.shape[0], np.int64)
    elif rep.size < x.shape[0]:
        rep = np.concatenate(
            [rep, np.full(x.shape[0] - rep.size, rep[-1], np.int64)]
        )
    for t in range(x.shape[0]):
        e, a = rep[t], adapter[t]
        inter = x[t] @ lora_A[a, e]
        out[t] += lora_scaling[a] * (inter @ lora_B[a, e])
    return out


def kernel(x, group_sizes, adapter_indices_sorted, weight, lora_A, lora_B, lora_scaling):
    import ml_dtypes

    x = np.ascontiguousarray(np.asarray(x, np.float32))
    weight = np.asarray(weight, np.float32)
    lora_A = np.asarray(lora_A, np.float32)
    lora_B = np.asarray(lora_B, np.float32)
    scaling = np.asarray(lora_scaling, np.float32)
    gs = np.asarray(group_sizes).astype(np.int64)
    adapter = np.asarray(adapter_indices_sorted).astype(np.int64)

    if gs.sum() != T or (gs < 0).any():
        return _reference_numpy(
            x, gs, adapter, weight, lora_A, lora_B, scaling
        )

    from concourse.bass_utils import run_bass_kernel_spmd

    bf = ml_dtypes.bfloat16
    f8 = ml_dtypes.float8_e4m3
    cap = int(max(128, -(-int(gs.max()) // 128) * 128))
    nc = _get_compiled(cap)

    offs = np.concatenate([[0], np.cumsum(gs)])
    in_maps = []
    for e in range(NCORES):
        n = int(gs[e])
        s = int(offs[e])
        ngr = (cap + 511) // 512
        xe = np.zeros((ngr * 512, IN), np.float32)
        xe[:n] = x[s : s + n]
        # [ngr, KC, 128, 512]: XT[g, k, p, t] = x_e[512g+t, 128k+p]
        xe_t = xe.T.reshape(KC, 128, ngr, 512).transpose(2, 0, 1, 3)
        xt = np.ascontiguousarray(xe_t.astype(bf))
        xt8 = np.ascontiguousarray(xe_t.astype(f8))
        w = np.ascontiguousarray(weight[e].reshape(KC, 128, OUT).astype(bf))
        # A_cat[:, a*R+r] = lora_A[a, e, :, r] -> [128, KC, AR], scaled by
        # 64 to land in fp8 e4m3's normal range; the mask divides it out.
        acat_full = lora_A[:, e].transpose(1, 0, 2).reshape(IN, AR)
        a8 = np.ascontiguousarray(
            (acat_full.reshape(KC, 128, AR) * 64.0).transpose(1, 0, 2).astype(f8)
        )
        bcat = np.ascontiguousarray(lora_B[:, e].reshape(AR, OUT).astype(bf))
        ae = adapter[s : s + n]
        m = np.zeros((A, cap), np.float32)
        m[ae, np.arange(n)] = scaling[ae] / 64.0
        maskt = np.ascontiguousarray(np.repeat(m, R, axis=0).astype(bf))
        in_maps.append(
            {"xt": xt, "xt8": xt8, "w": w, "a8": a8, "bcat": bcat, "maskt": maskt}
        )

    res = run_bass_kernel_spmd(nc, in_maps, list(range(NCORES)))

    out = np.empty((T, OUT), np.float32)
    for e in range(NCORES):
        n = int(gs[e])
        if n:
            out[int(offs[e]) : int(offs[e]) + n] = (
                res.results[e]["out"][:n].astype(np.float32)
            )
    return out


# revision 16
# speedup vs baseline: 1.1637x; 1.1003x over previous
"""Trainium2 Bass kernel for nn_LoRAExpert (moe_routing).

Per token t (expert e_t from contiguous group_sizes, adapter a_t):

    out[t] = x[t] @ W[e_t] + s_{a_t} * (x[t] @ A[a_t, e_t]) @ B[a_t, e_t]

Strategy (expert-parallel over 8 NeuronCores):
  - Host routes tokens: x is already expert-sorted, so core e gets the
    contiguous slice x[off_e : off_e + gs_e], padded to a common `cap`.
  - LoRA routing trick: with A=8 adapters and rank R=16, the per-expert
    concatenation A_cat = [A[0,e] .. A[7,e]] is [1024, 128]. Compute
    inter_all = x @ A_cat densely for ALL adapters, then multiply by a
    per-token mask M[j, t] = s_{a_t} * (j in adapter-a_t block) and feed
    the masked inter into B_cat = [B[0,e]; ..; B[7,e]] ([128, 1024]).
    This turns the ragged adapter grouping into two dense matmuls and
    one elementwise mask — no on-device sorting or control flow.
  - The B-side matmul accumulates into the same PSUM tile as the base
    matmul, so base + lora is free.
  - All matmul operands are cast to bf16 on the host (fp32 PSUM
    accumulation on the PE); output is fp32.

Timeline engineering (the steady state is at the PE roofline; the wins
are all in the lead-in and the tail):
  - The runtime NEFF prelude (engine barrier + base-addr TENSOR_LOADs)
    ends ~5.2us; nothing bass-side runs earlier.
  - Warm-up matmuls have NO data dependency (they read an uninitialized
    SBUF tile) so the PE starts ramping the HAM clock immediately at
    ~6us instead of waiting for the first DMA to land (~8.5us). The HAM
    grants full clock after ~3.5us of sustained PE activity; a multi-us
    PE idle during/after the ramp trips a 7-14us half-duty penalty
    window, so the warm-up count is sized to bridge until the first
    real operands (a8 + xt8 g0) have landed.
  - DMA enqueues are split across BOTH hardware-DGE engines (Sync and
    Activation) so the ~0.65us/enqueue serialization halves, and each
    queue's FIFO is ordered by first-use time. The scalar engine issues
    no activation ops (all PSUM->SBUF copies are on vector), so it
    never loads the activation table and can enqueue from ~6us.
  - First group runs k-major across THREE token tiles so one W k-chunk
    feeds 6 matmuls (~1.3us) — matching the per-queue W delivery rate
    while the other queue streams x/mask/bcat concurrently.
  - The bulk x groups are dep-chained behind most of W (a dummy vector
    op reading W k5 + the head of the not-yet-loaded xt region gives
    the bulk DMAs a WAR dependency) so they don't steal bandwidth from
    the critical path, and are enqueued per-group in consumption order.
  - Output DMAs ride the scalar queue; the last groups are split so the
    final transfer is one 128-token tile, keeping the tail short.
"""

import numpy as np

T, E, IN, OUT, A, R = 16384, 8, 1024, 1024, 8, 16
NCORES = 8
AR = A * R  # 128
KC = IN // 128  # 8 contraction chunks
OC = OUT // 512  # 2 output column chunks
WARMUP = 18  # sized so the warm-up ends ~= critical-x arrival (see notes)

_compiled_cache: dict[int, object] = {}


# ---------------------------------------------------------------------------
# walrus in this container accepts at most 1 sync-wait command per
# instruction; Tile attaches more. Split excess waits onto no-ops.
# ---------------------------------------------------------------------------


def _apply_tile_wait_patch():
    import bass_rust
    import concourse.tile as tile
    from concourse import mybir
    from concourse.vector_clock import ScopedClock

    if getattr(tile.TileContext, "_wait_split_patched", False):
        return

    MAX_WAITS = 1

    def _split_excess_waits(nc):
        for fn in nc.m.functions:
            for blk in fn.blocks:
                insts = blk.instructions  # live list
                i = 0
                while i < len(insts):
                    inst = insts[i]
                    si = inst.sync_info
                    if si is not None and len(si.on_wait) > MAX_WAITS:
                        waits = list(si.on_wait)
                        keep = waits[-MAX_WAITS:]
                        excess = waits[:-MAX_WAITS]
                        inst.sync_info = bass_rust.SyncInfo(
                            on_wait=keep, on_update=list(si.on_update)
                        )
                        pos = i
                        for k in range(0, len(excess), MAX_WAITS):
                            nop = mybir.InstNoOp(
                                name=f"{inst.name}-hoistw{k}",
                                engine=inst.engine,
                                bass_nofuse=True,
                                sync_info=mybir.SyncInfo(
                                    on_wait=excess[k : k + MAX_WAITS], on_update=[]
                                ),
                            )
                            insts.insert(pos, nop)
                            pos += 1
                            i += 1
                    i += 1

    def _split_drain_and_barrier(self, tick_clock, wait_clock):
        nc = self.nc
        drain_inst = nc.sync.drain()
        wait_clock.add_sem_waits(
            drain_inst.ins, ScopedClock({None: tick_clock.global_clock})
        )
        si = drain_inst.ins.sync_info
        if si is not None and len(si.on_wait) > MAX_WAITS:
            waits = list(si.on_wait)
            drain_inst.ins.sync_info = bass_rust.SyncInfo(
                on_wait=waits[:MAX_WAITS], on_update=list(si.on_update)
            )
            for k in range(MAX_WAITS, len(waits), MAX_WAITS):
                extra = nc.sync.drain()
                extra.ins.sync_info = bass_rust.SyncInfo(
                    on_wait=waits[k : k + MAX_WAITS], on_update=[]
                )

        import os as _os

        # Gather/release barrier: 2 sem hops (~1us) instead of the
        # 5-hop chained form (~2.5us).
        nc.all_engine_barrier(sem_only=True)
        assert self.sems is not None
        popped = nc._tile_sem_poison_stack.pop()
        assert popped is self._sem_poison
        nc.clear_and_free_semaphores(list(self.sems.allocated().values()))
        if _os.environ.get("LORA_LEAN_TAIL", "1") != "1":
            # Second barrier only matters for kernels that continue past
            # the TileContext; ours ends here (sem clears trail on gpsimd).
            nc.all_engine_barrier()

        _split_excess_waits(nc)

    tile.TileContext._drain_and_barrier = _split_drain_and_barrier
    tile.TileContext._wait_split_patched = True


# ---------------------------------------------------------------------------
# Bass program (one SPMD NeuronCore program, parameterized by cap)
# ---------------------------------------------------------------------------


def _build(cap: int):
    import concourse.bass as bass
    import concourse.tile as tile
    from concourse import mybir

    _apply_tile_wait_patch()

    ntt = cap // 128  # token tiles
    ngr = (cap + 511) // 512  # inter groups of up to 512 tokens

    bf16 = mybir.dt.bfloat16
    f32 = mybir.dt.float32
    f8e4 = mybir.dt.float8e4
    DR = mybir.MatmulPerfMode.DoubleRow

    # partition-id preamble and monotonic sems are unused here — skip them.
    # Also skip the single __init__ barrier (bass.py:7557): it only
    # orders the const-AP memsets, which nothing in this kernel reads.
    _orig_aeb = bass.Bass.all_engine_barrier
    bass.Bass.all_engine_barrier = lambda self, **kw: None
    try:
        nc = bass.Bass(enable_partition_id=False, monotonic_sem_count=0)
    finally:
        bass.Bass.all_engine_barrier = _orig_aeb
    # XT[g, k, p, c] = x_e[512g + c, 128k + p]
    XT = nc.dram_tensor("xt", [ngr, KC, 128, 512], bf16, kind="ExternalInput")
    XT8 = nc.dram_tensor("xt8", [ngr, KC, 128, 512], f8e4, kind="ExternalInput")
    W = nc.dram_tensor("w", [KC, 128, OUT], bf16, kind="ExternalInput")
    A8 = nc.dram_tensor("a8", [128, KC, AR], f8e4, kind="ExternalInput")
    BCAT = nc.dram_tensor("bcat", [AR, OUT], bf16, kind="ExternalInput")
    MASKT = nc.dram_tensor("maskt", [AR, cap], bf16, kind="ExternalInput")
    OUTD = nc.dram_tensor("out", [cap, OUT], bf16, kind="ExternalOutput")

    def gslice(g):
        t0 = g * 512
        return t0, min(512, cap - t0)

    # tiles in group 0 processed k-major in one fused block
    fancy = cap >= 512  # group 0 has 4 full tiles
    nfuse = 2

    with tile.TileContext(nc) as tc:
        with (
            tc.tile_pool(name="big", bufs=1) as big,
            tc.tile_pool(name="outp", bufs=2) as outp,
            tc.tile_pool(name="psi", bufs=2, space="PSUM") as psi,
            tc.tile_pool(name="pso", bufs=6, space="PSUM") as pso,
        ):
            # --- PE warm-up, phase A: no DMA dependency (reads mostly
            # uninitialized SBUF; the 1-column memset just allocates the
            # tile); output psum is a pso slot that later tiles reuse.
            warm_sb = big.tile([128, 4, AR], bf16)
            nc.vector.memset(warm_sb[:, 0, 0:1], 0)
            wps = pso.tile([128, 512], f32, name="pswarm", tag="pso")
            for i in range(WARMUP):
                nc.tensor.matmul(
                    wps[:], warm_sb[:, 0, :], warm_sb[:],
                    start=(i == 0), stop=(i == WARMUP - 1),
                )

            # --- DMA enqueues. Two HWDGE queues; concurrent queues
            # fair-share HBM read bandwidth, a solo queue gets nearly all
            # of it. The critical chain (x tiles 0-1 + W, which feed the
            # fused k-major block) rides the sync queue ALONE; everything
            # else rides the scalar queue, dep-chained behind W k0 so it
            # cannot steal bandwidth from the critical chain during the
            # DMA-fabric ramp.
            # xt_sb[p, g, k, c] = bf16 x for the base-matmul stationary
            xt_sb = big.tile([128, ngr, KC, 512], bf16)
            c0 = min(nfuse * 128, cap) if fancy else min(512, cap)
            w_sb = big.tile([128, KC, OUT], bf16)
            nc.sync.dma_start(
                xt_sb[:, 0, :, :c0],
                XT[0, :, :, :c0].rearrange("k p c -> p k c"),
            )
            for k in range(KC):
                nc.sync.dma_start(
                    w_sb[:, k : k + 1, :],
                    W[k : k + 1, :, :].rearrange("k p c -> p k c"),
                )

            a8_sb = big.tile([128, KC, AR], f8e4)
            xt8_sb = big.tile([128, ngr, KC, 512], f8e4)
            if fancy:
                # Gate the scalar queue behind the x head: dummy vector op
                # reading the landed xth head and the not-yet-written a8
                # region gives a8's DMA (and everything FIFO-behind it) a
                # WAR dependency on the critical x transfer.
                dep2_sb = big.tile([128, 1], f32)
                nc.vector.scalar_tensor_tensor(
                    dep2_sb[:],
                    w_sb[:, 0, OUT - 1 : OUT],
                    1.0,
                    a8_sb[:, 0, 0:1],
                    mybir.AluOpType.mult,
                    mybir.AluOpType.mult,
                )
            nc.scalar.dma_start(a8_sb[:], A8[:])
            # xt8_sb[p, g, k, c] = fp8(x_e[512g + c, 128k + p]); halves so
            # phase1's first DR matmuls start on the k0-3 half
            nc.scalar.dma_start(
                xt8_sb[:, 0, : KC // 2, :],
                XT8[0, : KC // 2, :, :].rearrange("k p c -> p k c"),
            )
            nc.scalar.dma_start(
                xt8_sb[:, 0, KC // 2 :, :],
                XT8[0, KC // 2 :, :, :].rearrange("k p c -> p k c"),
            )
            maskt_sb = big.tile([AR, cap], bf16)
            m0 = min(512, cap)
            if cap > c0:
                g0w = min(512, cap)
                nc.scalar.dma_start(
                    xt_sb[:, 0, :, c0:g0w],
                    XT[0, :, :, c0:g0w].rearrange("k p c -> p k c"),
                )
            nc.scalar.dma_start(maskt_sb[:, :m0], MASKT[:, :m0])
            b_sb = big.tile([AR, OUT], bf16)
            nc.scalar.dma_start(b_sb[:], BCAT[:])
            if cap > m0:
                nc.scalar.dma_start(maskt_sb[:, m0:], MASKT[:, m0:])

            if ngr > 1:
                # Dummy op READING W k5 and the head of the not-yet-loaded
                # xt region: the bulk x DMAs then carry a WAR dependency on
                # it, so they wait for most of W before competing for HBM
                # bandwidth. (A write INTO xt would race the DMA.)
                dep_sb = big.tile([128, 1], f32)
                nc.vector.scalar_tensor_tensor(
                    dep_sb[:],
                    w_sb[:, KC - 3, OUT - 1 : OUT],
                    1.0,
                    xt_sb[:, 1, 0, 0:1],
                    mybir.AluOpType.mult,
                    mybir.AluOpType.mult,
                )
                # bulk groups, per-group DMAs in consumption order
                for g in range(1, ngr):
                    nc.sync.dma_start(
                        xt8_sb[:, g, :, :],
                        XT8[g, :, :, :].rearrange("k p c -> p k c"),
                    )
                    nc.sync.dma_start(
                        xt_sb[:, g, :, :],
                        XT[g, :, :, :].rearrange("k p c -> p k c"),
                    )

            interm_sb = big.tile([AR, cap], bf16)

            def phase1(g):
                # inter_all = (x8 @ (64*A_cat))^T via fp8 DoubleRow (two
                # 128-deep k-chunks per instruction), masked by s_a/64
                # -> interm_sb
                t0, wg = gslice(g)
                ps = psi.tile([128, 512], f32, name=f"psi{g}", tag="psi")
                for i in range(KC // 2):
                    nc.tensor.matmul(
                        ps[:, :wg],
                        a8_sb[:, 2 * i : 2 * i + 2, :],
                        xt8_sb[:, g, 2 * i : 2 * i + 2, :wg],
                        start=(i == 0),
                        stop=(i == KC // 2 - 1),
                        perf_mode=DR,
                    )
                nc.vector.scalar_tensor_tensor(
                    interm_sb[:, t0 : t0 + wg],
                    ps[:, :wg],
                    1.0,
                    maskt_sb[:, t0 : t0 + wg],
                    mybir.AluOpType.mult,
                    mybir.AluOpType.mult,
                )

            def copies(tt, pss, o_sb):
                # psum -> output stage, oc0 on vector, oc1 on scalar
                j = tt % 4
                for oc in range(OC):
                    dst = o_sb[:, j * OUT + oc * 512 : j * OUT + oc * 512 + 512]
                    if oc == 0:
                        nc.vector.tensor_copy(dst, pss[oc][:])
                    else:
                        nc.scalar.copy(dst, pss[oc][:])

            def bside(tt, pss):
                ts0 = tt * 128
                for oc in range(OC):
                    nc.tensor.matmul(
                        pss[oc][:],
                        interm_sb[:, ts0 : ts0 + 128],
                        b_sb[:, oc * 512 : oc * 512 + 512],
                        start=False,
                        stop=True,
                    )

            def token_tile(tt, o_sb):
                # base k-loop into 2 psum banks (one per 512-wide output
                # chunk), + 1 lora matmul each, then copy to group stage.
                g, j = tt // 4, tt % 4
                pss = [
                    pso.tile([128, 512], f32, name=f"psod{tt}_{i}", tag="pso")
                    for i in range(OC)
                ]
                for k in range(KC):
                    for oc in range(OC):
                        nc.tensor.matmul(
                            pss[oc][:],
                            xt_sb[:, g, k, j * 128 : j * 128 + 128],
                            w_sb[:, k, oc * 512 : oc * 512 + 512],
                            start=(k == 0),
                            stop=False,
                        )
                bside(tt, pss)
                copies(tt, pss, o_sb)

            def out_dma(g, o_sb):
                t0, wg = gslice(g)
                ntg = wg // 128
                if g < ngr - 2:
                    # one output DMA per group: [128, ntg, OUT] rows
                    nc.scalar.dma_start(
                        OUTD[t0 : t0 + wg, :].rearrange(
                            "(j p) c -> p j c", p=128
                        ),
                        o_sb[:].rearrange("p (j c) -> p j c", c=OUT),
                    )
                elif g == ngr - 2 and ntg > 1:
                    # split the second-to-last group so its output mostly
                    # clears the wire before the final tile's transfer
                    h = ntg // 2
                    nc.scalar.dma_start(
                        OUTD[t0 : t0 + h * 128, :].rearrange(
                            "(j p) c -> p j c", p=128
                        ),
                        o_sb[:, : h * OUT].rearrange("p (j c) -> p j c", c=OUT),
                    )
                    nc.scalar.dma_start(
                        OUTD[t0 + h * 128 : t0 + wg, :].rearrange(
                            "(j p) c -> p j c", p=128
                        ),
                        o_sb[:, h * OUT :].rearrange("p (j c) -> p j c", c=OUT),
                    )
                else:
                    # last group: per-(tile, oc) DMAs so the final
                    # transfer starts right after its own copy instead of
                    # waiting for all of the group's copies.
                    for j in range(wg // 128):
                        for oc in range(OC):
                            nc.scalar.dma_start(
                                OUTD[
                                    t0 + j * 128 : t0 + j * 128 + 128,
                                    oc * 512 : oc * 512 + 512,
                                ],
                                o_sb[
                                    :,
                                    j * OUT + oc * 512 : j * OUT + oc * 512 + 512,
                                ],
                            )

            if fancy:
                # ---- group 0: fused k-major over tiles 0-1 (their x head
                # is the 0.5MB critical transfer the warm-up bridges to),
                # with phase1(0) and tile 2's base k-loop woven in so the
                # PE never waits on the (late, scalar-queue) A-side
                # operands or the vector mask STT.
                o_sb = outp.tile([128, 4 * OUT], bf16, name="og0", tag="outp")
                pss = [
                    pso.tile([128, 512], f32, name=f"psod{t}_{i}", tag="pso")
                    for t in range(nfuse)
                    for i in range(OC)
                ]
                for k in range(KC):
                    for t in range(nfuse):
                        for oc in range(OC):
                            nc.tensor.matmul(
                                pss[t * OC + oc][:],
                                xt_sb[:, 0, k, t * 128 : t * 128 + 128],
                                w_sb[:, k, oc * 512 : oc * 512 + 512],
                                start=(k == 0),
                                stop=False,
                            )
                phase1(0)
                # tile 2 base between phase1(0) and the B-sides: 3.4us of
                # dependency-free PE work while the mask STT completes.
                ps2 = [
                    pso.tile([128, 512], f32, name=f"psod2_{i}", tag="pso")
                    for i in range(OC)
                ]
                for k in range(KC):
                    for oc in range(OC):
                        nc.tensor.matmul(
                            ps2[oc][:],
                            xt_sb[:, 0, k, 256:384],
                            w_sb[:, k, oc * 512 : oc * 512 + 512],
                            start=(k == 0),
                            stop=False,
                        )
                bside(0, pss[0:2])
                copies(0, pss[0:2], o_sb)
                bside(1, pss[2:4])
                copies(1, pss[2:4], o_sb)
                bside(2, ps2)
                copies(2, ps2, o_sb)
                token_tile(3, o_sb)
                if ngr > 1:
                    phase1(1)
                out_dma(0, o_sb)
                g_start = 1
            else:
                phase1(0)
                g_start = 0

            for g in range(g_start, ngr):
                t0, wg = gslice(g)
                ntg = wg // 128
                # group output stage: o_sb[p, j*OUT + c] = out[t0+128j+p, c]
                o_sb = outp.tile(
                    [128, ntg * OUT], bf16, name=f"og{g}", tag="outp"
                )
                tts = list(range(t0 // 128, (t0 + wg) // 128))
                for j, tt in enumerate(tts):
                    token_tile(tt, o_sb)
                    # emit next group's phase 1 after this group's LAST
                    # token tile: by then its (dep-chained, late) x8 chunk
                    # has arrived, and the mask STT still completes before
                    # the first B-side matmul of group g+1 needs it.
                    if j == len(tts) - 1 and g + 1 < ngr:
                        phase1(g + 1)
                out_dma(g, o_sb)

    return nc


def _get_compiled(cap: int):
    if cap not in _compiled_cache:
        _compiled_cache[cap] = _build(cap)
    return _compiled_cache[cap]


# ---------------------------------------------------------------------------
# Host-side routing + execution
# ---------------------------------------------------------------------------


def _reference_numpy(x, group_sizes, adapter_indices_sorted, weight, lora_A, lora_B, lora_scaling):
    """Fallback replicating the jax reference exactly (only used for
    degenerate group_sizes that do not sum to T)."""
    x = np.asarray(x, np.float32)
    gs = np.asarray(group_sizes, np.int64)
    adapter = np.asarray(adapter_indices_sorted, np.int64)
    out = np.zeros((x.shape[0], weight.shape[2]), np.float32)
    # base: ragged_dot semantics (groups from cumsum, tail rows -> 0)
    offs = np.minimum(np.concatenate([[0], np.cumsum(gs)]), x.shape[0])
    for e in range(E):
        s, t = offs[e], offs[e + 1]
        if t > s:
            out[s:t] = x[s:t] @ weight[e]
    # lora: expert ids via repeat padded with the final value
    rep = np.repeat(np.arange(E), np.maximum(gs, 0))[: x.shape[0]]
    if rep.size == 0:
        rep = np.zeros(x.shape[0], np.int64)
    elif rep.size < x.shape[0]:
        rep = np.concatenate(
            [rep, np.full(x.shape[0] - rep.size, rep[-1], np.int64)]
        )
    for t in range(x.shape[0]):
        e, a = rep[t], adapter[t]
        inter = x[t] @ lora_A[a, e]
        out[t] += lora_scaling[a] * (inter @ lora_B[a, e])
    return out


def kernel(x, group_sizes, adapter_indices_sorted, weight, lora_A, lora_B, lora_scaling):
    import ml_dtypes

    x = np.ascontiguousarray(np.asarray(x, np.float32))
    weight = np.asarray(weight, np.float32)
    lora_A = np.asarray(lora_A, np.float32)
    lora_B = np.asarray(lora_B, np.float32)
    scaling = np.asarray(lora_scaling, np.float32)
    gs = np.asarray(group_sizes).astype(np.int64)
    adapter = np.asarray(adapter_indices_sorted).astype(np.int64)

    if gs.sum() != T or (gs < 0).any():
        return _reference_numpy(
            x, gs, adapter, weight, lora_A, lora_B, scaling
        )

    from concourse.bass_utils import run_bass_kernel_spmd

    bf = ml_dtypes.bfloat16
    f8 = ml_dtypes.float8_e4m3
    cap = int(max(128, -(-int(gs.max()) // 128) * 128))
    nc = _get_compiled(cap)

    offs = np.concatenate([[0], np.cumsum(gs)])
    in_maps = []
    for e in range(NCORES):
        n = int(gs[e])
        s = int(offs[e])
        ngr = (cap + 511) // 512
        xe = np.zeros((ngr * 512, IN), np.float32)
        xe[:n] = x[s : s + n]
        # [ngr, KC, 128, 512]: XT[g, k, p, t] = x_e[512g+t, 128k+p]
        xe_t = xe.T.reshape(KC, 128, ngr, 512).transpose(2, 0, 1, 3)
        xt = np.ascontiguousarray(xe_t.astype(bf))
        xt8 = np.ascontiguousarray(xe_t.astype(f8))
        w = np.ascontiguousarray(weight[e].reshape(KC, 128, OUT).astype(bf))
        # A_cat[:, a*R+r] = lora_A[a, e, :, r] -> [128, KC, AR], scaled by
        # 64 to land in fp8 e4m3's normal range; the mask divides it out.
        acat_full = lora_A[:, e].transpose(1, 0, 2).reshape(IN, AR)
        a8 = np.ascontiguousarray(
            (acat_full.reshape(KC, 128, AR) * 64.0).transpose(1, 0, 2).astype(f8)
        )
        bcat = np.ascontiguousarray(lora_B[:, e].reshape(AR, OUT).astype(bf))
        ae = adapter[s : s + n]
        m = np.zeros((A, cap), np.float32)
        m[ae, np.arange(n)] = scaling[ae] / 64.0
        maskt = np.ascontiguousarray(np.repeat(m, R, axis=0).astype(bf))
        in_maps.append(
            {"xt": xt, "xt8": xt8, "w": w, "a8": a8, "bcat": bcat, "maskt": maskt}
        )

    res = run_bass_kernel_spmd(nc, in_maps, list(range(NCORES)))

    out = np.empty((T, OUT), np.float32)
    for e in range(NCORES):
        n = int(gs[e])
        if n:
            out[int(offs[e]) : int(offs[e]) + n] = (
                res.results[e]["out"][:n].astype(np.float32)
            )
    return out
